# revision 28
# baseline (speedup 1.0000x reference)
"""MoE feed-forward (B=4,S=2048,D=1024,F=2048,E=8,top-2) on 8 trn2 NeuronCores.

Strategy (F-split tensor parallel — perfectly load balanced):
 - Host computes top-2 softmax routing and builds the expert-sorted column
   list (16384 token-expert pairs).
 - Core j owns F-slice [j*256, (j+1)*256) of every expert: W1/W3 column
   slices and the matching W2 row slice.  Every core processes ALL 16384
   columns — identical layout on all cores, so the SPMD program is the same
   everywhere and per-core work is exactly balanced regardless of routing.
 - Columns are tiled per expert into near-equal blocks of <=512 (PSUM
   width), paired into chunks of <=1024 (x/y DMA granularity).  Every block
   is single-expert, so no boundary-split matmuls and no tiny-block waste.
 - All HBM traffic uses host-packed, partition-contiguous layouts so each
   DMA is 128 descriptors of 4-16 KB (the DMA queues are descriptor-rate
   bound at ~85ns/descriptor, so 512B-descriptor transfers stall the PE).
   Weights stream one packed DMA per expert, just ahead of that expert's
   blocks.
 - Per block: h1 = W1_e^T x (2 f-tiles x 8 k), s = silu(h1), h3 = W3_e^T x,
   g = s*h3, y_partial = W2_e^T g (8 d-tiles x 2 kf, kf-outer in groups of
   4 d-tiles so early matmuls hide the g-multiply latency).
 - y partials (fp16) are summed over the 8 cores on the host, which also
   applies the top-2 combine weights in fp32 during the scatter-add.
"""

import numpy as np

import concourse.tile as tile
from concourse import bacc, mybir
from concourse.bass_utils import run_bass_kernel_spmd

B, S, D, F, E, TOPK = 4, 2048, 1024, 2048, 8, 2
N_CORES = 8
KD = D // 128          # 8 contraction tiles for D
FS = F // N_CORES      # 256-wide F slice per core
NF = FS // 128         # 2 f-tiles per expert per core
KW = FS // 128         # 2 contraction tiles for the W2 pass
W3OFF = KD * FS        # offsets into the packed per-expert weight slab
W2OFF = 2 * KD * FS
WCOLS = 2 * KD * FS + KW * D   # 6144 fp16 per partition per expert
XCOLS = KD * 1024              # flat x/y chunk tile width

_nc_cache = {}


def _expert_order(counts):
    """Experts in column order: an odd-block expert goes last so the kernel
    can end on a single small block (short tail flush)."""
    live = [e for e in range(len(counts)) if counts[e] > 0]
    odd = [e for e in live if (-(-counts[e] // 512)) % 2 == 1]
    if not odd:
        return live
    last = odd[-1]
    return [e for e in live if e != last] + [last]


def _layout(counts):
    """Per-expert near-equal blocks paired into chunks.

    Returns a list of (expert, col0, [block sizes]) chunks in column order.
    The final expert's last block is forced small (if its block count is
    odd) so the tail writeback is tiny.
    """
    order = _expert_order(counts)
    chunks = []
    col = 0
    for oi, e in enumerate(order):
        c = counts[e]
        nb = -(-c // 512)
        if oi == len(order) - 1 and nb % 2 == 1 and nb >= 3:
            s_last = max(140, c - (nb - 1) * 512)
            rest = c - s_last
            base, extra = divmod(rest, nb - 1)
            sizes = [base + (1 if i < extra else 0) for i in range(nb - 1)]
            sizes.append(s_last)
        else:
            base, extra = divmod(c, nb)
            sizes = [base + (1 if i < extra else 0) for i in range(nb)]
        off = 0
        if oi == len(order) - 1:
            # single-block chunks: each writeback issues a whole block
            # before the kernel tail instead of bunching at the end
            for sz in sizes:
                chunks.append((e, col + off, [sz]))
                off += sz
        else:
            for i in range(0, len(sizes), 2):
                blks = sizes[i:i + 2]
                chunks.append((e, col + off, blks))
                off += sum(blks)
        col += c
    return chunks


def _build_nc(counts):
    """Per-core Bass program; counts = tokens per expert."""
    f16 = mybir.dt.float16
    f32 = mybir.dt.float32
    chunks = _layout(counts)
    NCH = len(chunks)

    nc = bacc.Bacc(None, target_bir_lowering=False, enable_partition_id=False,
                   monotonic_sem_count=0)
    # Host-packed layouts: partition-contiguous per chunk / per expert.
    xTp = nc.dram_tensor("xTp", [NCH * 128, XCOLS], f16, kind="ExternalInput")
    Wall = nc.dram_tensor("Wall", [E * 128, WCOLS], f16, kind="ExternalInput")
    yTp = nc.dram_tensor("yTp", [NCH * 128, XCOLS], f16, kind="ExternalOutput")

    experts = [e for e in range(E) if counts[e] > 0]

    with tile.TileContext(nc) as tc:
        with (
            tc.tile_pool(name="wpool", bufs=1) as wpool,
            tc.tile_pool(name="xpool", bufs=3) as xpool,
            tc.tile_pool(name="gpool", bufs=4) as gpool,
            tc.tile_pool(name="spool", bufs=4) as spool,
            tc.tile_pool(name="ypool", bufs=2) as ypool,
            tc.tile_pool(name="ps1", bufs=2, space="PSUM") as ps1p,
            tc.tile_pool(name="ps3", bufs=2, space="PSUM") as ps3p,
            tc.tile_pool(name="psY", bufs=4, space="PSUM") as psYp,
        ):
            # Per-expert packed weight slabs, all SBUF-resident (12 KB/part)
            wall = {}
            for e in experts:
                wall[e] = wpool.tile([128, WCOLS], f16, tag=f"we{e}",
                                     name=f"we{e}")

            def load_weights(e):
                r = slice(e * 128, (e + 1) * 128)
                nc.sync.dma_start(out=wall[e], in_=Wall[r, :])

            def w1_ap(e, k, f):
                c = k * FS + f * 128
                return wall[e][:, c:c + 128]

            def w3_ap(e, k, f):
                c = W3OFF + k * FS + f * 128
                return wall[e][:, c:c + 128]

            def w2_ap(e, kf, dd):
                c = W2OFF + kf * D + dd * 128
                return wall[e][:, c:c + 128]

            # Preamble: first x k-row + first expert's W1, then the rest.
            e0 = chunks[0][0]
            n0 = sum(chunks[0][2])
            xs = [None] * NCH
            xs[0] = xpool.tile([128, XCOLS], f16, tag="x", name="x0")
            # Deadline-ordered preamble: dma_start issue is serialized on the
            # sync engine (~0.6us each), so emit in the order the pass-1
            # k-loop consumes — x rows front-loaded, W1 halves at their k4
            # deadline, W3/W2 after (pass 2/3 need them much later).
            r0 = slice(e0 * 128, (e0 + 1) * 128)
            def _xrow(k):
                nc.sync.dma_start(out=xs[0][:, k * n0:(k + 1) * n0],
                                  in_=xTp[0:128, k * n0:(k + 1) * n0])
            def _wsec(a, b):
                nc.sync.dma_start(out=wall[e0][:, a:b], in_=Wall[r0, a:b])
            _xrow(0)
            _wsec(0, W3OFF // 2)
            for k in range(1, 4):
                _xrow(k)
            _xrow(4)
            _wsec(W3OFF // 2, W3OFF)
            for k in range(5, KD):
                _xrow(k)
            _wsec(W3OFF, W3OFF + KD * FS // 2)
            _wsec(W3OFF + KD * FS // 2, W2OFF)
            _wsec(W2OFF, W2OFF + KW * D // 2)
            _wsec(W2OFF + KW * D // 2, WCOLS)

            for ci, (e, col0, blks) in enumerate(chunks):
                n = sum(blks)
                # prefetch next chunk's x
                if ci + 1 < NCH:
                    nn = sum(chunks[ci + 1][2])
                    xs[ci + 1] = xpool.tile([128, XCOLS], f16, tag="x",
                                            name=f"x{ci + 1}")
                    nc.sync.dma_start(
                        out=xs[ci + 1][:, :KD * nn],
                        in_=xTp[(ci + 1) * 128:(ci + 2) * 128, :KD * nn])
                # prefetch the next expert's weights at the start of this
                # expert's final chunk
                if ci + 1 == NCH or chunks[ci + 1][0] != e:
                    enext = chunks[ci + 1][0] if ci + 1 < NCH else None
                    if enext is not None:
                        load_weights(enext)

                xsb = xs[ci]
                ysb = ypool.tile([128, XCOLS], f16, tag="y", name=f"y{ci}")

                offs = [sum(blks[:i]) for i in range(len(blks))]
                sts = [None] * len(blks)
                gts = [None] * len(blks)

                def pass1(bi, e=e, ci=ci, xsb=xsb, n=n, blks=blks, offs=offs,
                          sts=sts):
                    nb, off = blks[bi], offs[bi]
                    sts[bi] = []
                    for f in range(NF):
                        ps1 = ps1p.tile([128, 512], f32, tag="ps1",
                                        name=f"ps1_{ci}_{bi}_{f}")
                        for k in range(KD):
                            nc.tensor.matmul(
                                ps1[:, :nb], lhsT=w1_ap(e, k, f),
                                rhs=xsb[:, k * n + off:k * n + off + nb],
                                start=(k == 0), stop=(k == KD - 1),
                            )
                        s = spool.tile([128, 512], f16, tag=f"s{f}",
                                       name=f"s_{ci}_{bi}_{f}")
                        nc.scalar.activation(s[:, :nb], ps1[:, :nb],
                                             mybir.ActivationFunctionType.Silu)
                        sts[bi].append(s)

                def pass2(bi, e=e, ci=ci, xsb=xsb, n=n, blks=blks, offs=offs,
                          sts=sts, gts=gts):
                    nb, off = blks[bi], offs[bi]
                    gts[bi] = []
                    for f in range(NF):
                        ps3 = ps3p.tile([128, 512], f32, tag="ps3",
                                        name=f"ps3_{ci}_{bi}_{f}")
                        for k in range(KD):
                            nc.tensor.matmul(
                                ps3[:, :nb], lhsT=w3_ap(e, k, f),
                                rhs=xsb[:, k * n + off:k * n + off + nb],
                                start=(k == 0), stop=(k == KD - 1),
                            )
                        g = gpool.tile([128, 512], f16, tag=f"g{f}",
                                       name=f"g_{ci}_{bi}_{f}")
                        nc.vector.tensor_mul(g[:, :nb], sts[bi][f][:, :nb],
                                             ps3[:, :nb])
                        gts[bi].append(g)

                def pass3(bi, e=e, ci=ci, ysb=ysb, n=n, blks=blks, offs=offs,
                          gts=gts):
                    # kf-outer over groups of 4 d-tiles so the first kf=0
                    # matmuls (which only need g0) cover the g1 mul latency
                    nb, off = blks[bi], offs[bi]
                    for g0 in range(0, KD, 4):
                        psys = [
                            psYp.tile([128, 512], f32, tag="psy",
                                      name=f"psy_{ci}_{bi}_{g0}_{i}")
                            for i in range(4)
                        ]
                        for kf in range(KW):
                            for i in range(4):
                                nc.tensor.matmul(
                                    psys[i][:, :nb],
                                    lhsT=w2_ap(e, kf, g0 + i),
                                    rhs=gts[bi][kf][:, :nb],
                                    start=(kf == 0), stop=(kf == KW - 1),
                                )
                        for i in range(4):
                            dd = g0 + i
                            dst = ysb[:, dd * n + off:dd * n + off + nb]
                            if dd % 2 == 0:
                                nc.scalar.copy(dst, psys[i][:, :nb])
                            else:
                                nc.vector.tensor_copy(dst, psys[i][:, :nb])

                if ci == 0 and len(blks) > 1:
                    # pass-major over the first chunk: pass-1 compute covers
                    # the arrival of the W3/W2 weight sections
                    for bi in range(len(blks)):
                        pass1(bi)
                    for bi in range(len(blks)):
                        pass2(bi)
                    for bi in range(len(blks)):
                        pass3(bi)
                else:
                    for bi in range(len(blks)):
                        pass1(bi)
                        pass2(bi)
                        pass3(bi)

                r = slice(ci * 128, (ci + 1) * 128)
                if ci == NCH - 2:
                    # split so the first half issues as soon as d-tiles 0-3
                    # drain — keeps the writeback off the kernel tail
                    h = (KD // 2) * n
                    nc.sync.dma_start(out=yTp[r, :h], in_=ysb[:, :h])
                    nc.sync.dma_start(out=yTp[r, h:KD * n], in_=ysb[:, h:KD * n])
                elif ci == NCH - 1:
                    # per-d-tile flush overlaps the final block's own compute
                    for dd in range(KD):
                        nc.sync.dma_start(out=yTp[r, dd * n:(dd + 1) * n],
                                          in_=ysb[:, dd * n:(dd + 1) * n])
                else:
                    nc.sync.dma_start(out=yTp[r, :KD * n], in_=ysb[:, :KD * n])
    nc.finalize()
    return nc


def _route(x, Wg):
    """Top-2 softmax routing in float64 (matches the f32 reference selection)."""
    logits = x.astype(np.float64) @ Wg.astype(np.float64)
    logits -= logits.max(axis=-1, keepdims=True)
    g = np.exp(logits)
    g /= g.sum(axis=-1, keepdims=True)
    top_i = np.argpartition(-g, TOPK - 1, axis=-1)[:, :TOPK]      # [T, 2]
    tg = np.take_along_axis(g, top_i, axis=-1)
    tg = tg / tg.sum(axis=-1, keepdims=True)
    return top_i, tg


def run(inputs, trace=False, trace_cores=None):
    hidden_states = np.asarray(inputs["hidden_states"], dtype=np.float32)
    Wg = np.asarray(inputs["Wg"], dtype=np.float32)
    W1 = np.asarray(inputs["W1"], dtype=np.float32)
    W3 = np.asarray(inputs["W3"], dtype=np.float32)
    W2 = np.asarray(inputs["W2"], dtype=np.float32)

    x = hidden_states.reshape(-1, D)                              # [T, D]
    T = x.shape[0]
    top_i, tg = _route(x, Wg)

    idx = []
    wts = []
    for e in range(E):
        sel = top_i == e                                          # [T, 2]
        rows = np.where(sel.any(axis=-1))[0]
        idx.append(rows)
        wts.append(np.where(sel[rows, 0], tg[rows, 0], tg[rows, 1]))
    counts = tuple(len(r) for r in idx)
    assert sum(counts) == T * TOPK

    if counts not in _nc_cache:
        _nc_cache[counts] = _build_nc(counts)
    nc = _nc_cache[counts]

    chunks = _layout(counts)
    NCH = len(chunks)
    eorder = _expert_order(counts)
    order = np.concatenate([idx[e] for e in eorder])              # [sum C_e]
    xT16 = x.T.astype(np.float16)                                 # [D, T]

    # Pack x: per chunk, [128 partitions, KD*n] partition-contiguous
    xTp = np.zeros((NCH * 128, XCOLS), np.float16)
    for ci, (e, col0, blks) in enumerate(chunks):
        n = sum(blks)
        sel = order[col0:col0 + n]
        xc = xT16[:, sel].reshape(KD, 128, n).transpose(1, 0, 2)
        xTp[ci * 128:(ci + 1) * 128, :KD * n] = xc.reshape(128, KD * n)

    in_maps = []
    for j in range(N_CORES):
        cs = slice(j * FS, (j + 1) * FS)
        Wc = np.zeros((E * 128, WCOLS), np.float16)
        for e in range(E):
            w1 = W1[e][:, cs].astype(np.float16)                  # [D, FS]
            w3 = W3[e][:, cs].astype(np.float16)
            w2 = W2[e][cs, :].astype(np.float16)                  # [FS, D]
            r = slice(e * 128, (e + 1) * 128)
            Wc[r, :W3OFF] = w1.reshape(KD, 128, FS).transpose(1, 0, 2) \
                              .reshape(128, KD * FS)
            Wc[r, W3OFF:W2OFF] = w3.reshape(KD, 128, FS).transpose(1, 0, 2) \
                                   .reshape(128, KD * FS)
            Wc[r, W2OFF:] = w2.reshape(KW, 128, D).transpose(1, 0, 2) \
                              .reshape(128, KW * D)
        in_maps.append({"xTp": xTp, "Wall": Wc})

    kwargs = {}
    if trace:
        kwargs["trace"] = True
        kwargs["trace_cores"] = trace_cores or list(range(N_CORES))
    res = run_bass_kernel_spmd(nc, in_maps, list(range(N_CORES)), **kwargs)

    ysum = res.results[0]["yTp"].astype(np.float32)
    for j in range(1, N_CORES):
        ysum += res.results[j]["yTp"].astype(np.float32)          # packed

    # Unpack per chunk into [D, C] column order, then combine on the host
    yT = np.empty((D, T * TOPK), np.float32)
    for ci, (e, col0, blks) in enumerate(chunks):
        n = sum(blks)
        yc = ysum[ci * 128:(ci + 1) * 128, :KD * n].reshape(128, KD, n)
        yT[:, col0:col0 + n] = yc.transpose(1, 0, 2).reshape(D, n)

    out = np.zeros((T, D), np.float32)
    lo = 0
    for e in eorder:
        c = counts[e]
        out[idx[e]] += wts[e][:, None].astype(np.float32) * yT[:, lo:lo + c].T
        lo += c
    return out.reshape(B, S, D), res


def kernel(**inputs):
    out, _ = run(inputs, trace=False)
    return out
